# revision 15
# baseline (speedup 1.0000x reference)
"""Trainium2 Bass kernel for batched attention:
    out[b] = softmax(q[b] @ m[b].T / 0.02) @ m[b]
with q, m: [8, 2048, 1024] fp32.

Sharding: data-parallel over batch — core b computes batch element b.

Numerics: the softmax temperature (x50) makes logits huge (std ~1600), so
softmax is near-one-hot and the scores matmul needs ~fp32 precision to keep
the argmax/mixing stable. Native fp32 matmul costs 4 PE-cycles/row; instead
mm1 uses an fp16 hi/lo 3-pass split (qh*mh + qh*ml + ql*mh, fp32 PSUM
accumulation) at 3 cycles/row with ~22-bit effective mantissa — measured
absmax relative error ~1e-3 vs fp64, same envelope as a pure-fp32 pipeline.
mm2 (P @ M) is plain fp16: P's dominant weights are exactly representable
and fp16(M)'s 11-bit mantissa matches what the hardware's tf32 mode would
keep anyway.

Per-core dataflow (Lq=Lkv=2048, D=1024, q-tile = 128 rows):
  setup:  mh_nat [128,16,1024] f16  = fp16(M) by k-chunk  (mm2 rhs)
          MhT    [128,8,2048]  f16  = fp16(M)^T by d-chunk  (PE transposes)
          MlT    [128,8,2048]  f16  = (M - fp16(M))^T
  per q-tile:
          qh/ql  [128,8,128]   f16  = hi/lo of 50*Q_tile, transposed
          S      = 3-pass matmul -> PSUM [128, 4x512] f32
          P      = exp(S - rowmax) -> SBUF f16 (ACT; accum_out = row sums)
          PT     [128,16,128]  f16  = P^T (PE transposes)
          O      = PT.T @ mh_nat -> PSUM [128,1024] f32
          out    = O * (1/rowsum) -> f32 -> DMA out
"""

import sys

if "/opt/trn_rl_repo" not in sys.path:
    sys.path.insert(0, "/opt/trn_rl_repo")

import numpy as np

B = 8
LQ = 2048
LKV = 2048
D = 1024
P = 128
NQT = LQ // P       # 16 q tiles
NKC = LKV // P      # 16 k chunks
NDC = D // P        # 8 d chunks
NS1 = LKV // 512    # 4 n-slices for mm1 (one PSUM bank each)
NS2 = D // 512      # 2 n-slices for mm2
SCALE = 1.0 / 0.02  # 50.0

_CACHE = {}


def _patch_json(nc):
    """This container's walrus supports only ONE sync-wait per instruction.
    Split any multi-wait instruction into preceding single-wait Drains on
    the same engine (engines execute in order, so semantics are identical)."""
    import orjson

    orig = nc.to_json_bytes

    def fixed():
        d = orjson.loads(orig())
        for fn in d["functions"]:
            for bb in fn["blocks"]:
                new = []
                for inst in bb.get("instructions", []):
                    si = inst.get("sync_info") or {}
                    ow = si.get("on_wait") or []
                    if len(ow) > 1:
                        excess, keep = ow[:-1], ow[-1:]
                        si["on_wait"] = keep
                        for k, w in enumerate(excess):
                            new.append({
                                "debug": inst.get("debug", 0),
                                "engine": inst["engine"],
                                "ins": [], "outs": [],
                                "is_reset_sema": False,
                                "name": f"{inst['name']}-sw{k}",
                                "opcode": "Drain",
                                "sync_info": {"on_update": [], "on_wait": [w]},
                            })
                    new.append(inst)
                bb["instructions"] = new
        return orjson.dumps(d)

    nc.to_json_bytes = fixed
    return nc


def build_nc():
    import concourse.bass as bass
    import concourse.mybir as mybir
    import concourse.tile as tile
    from concourse.masks import make_identity

    f32 = mybir.dt.float32
    f16 = mybir.dt.float16
    AX = mybir.AxisListType.X
    EXP = mybir.ActivationFunctionType.Exp

    nc = bass.Bass()
    q_d = nc.dram_tensor("q", [LQ, D], f32, kind="ExternalInput")
    m_d = nc.dram_tensor("m", [LKV, D], f32, kind="ExternalInput")
    o_d = nc.dram_tensor("out", [LQ, D], f32, kind="ExternalOutput")

    q_ap = q_d.ap()
    m_ap = m_d.ap()
    o_ap = o_d.ap()

    with tile.TileContext(nc) as tc:
        with (
            tc.tile_pool(name="const", bufs=1) as const_pool,
            tc.tile_pool(name="mres", bufs=1) as mres_pool,
            tc.tile_pool(name="qload", bufs=3) as qload_pool,
            tc.tile_pool(name="qsplit", bufs=2) as qsplit_pool,
            tc.tile_pool(name="qt", bufs=2) as qt_pool,
            tc.tile_pool(name="psb", bufs=2) as p_pool,
            tc.tile_pool(name="ptt", bufs=2) as pt_pool,
            tc.tile_pool(name="osb", bufs=3) as out_pool,
            tc.tile_pool(name="vec", bufs=6) as vec_pool,
            tc.tile_pool(name="msplit", bufs=3) as msplit_pool,
            tc.tile_pool(name="ps_s", bufs=1, space="PSUM") as ps_s,
            tc.tile_pool(name="ps_o", bufs=1, space="PSUM") as ps_o,
            tc.tile_pool(name="ps_t", bufs=2, space="PSUM") as ps_t,
        ):
            ident16 = const_pool.tile([P, P], f16)
            make_identity(nc, ident16)

            # ---- resident M derivatives: mh_nat (f16, natural), MhT/MlT
            # (f16, transposed by d-chunk).
            # Transposes land in grouped [128, 4x128] PSUM tiles so ONE
            # [128,512] copy moves four transposed blocks to SBUF.
            # Note: a transposed [d, k] block of chunk kc for d-chunk dc sits
            # at mht[:, dc, kc*128:(kc+1)*128] — the four blocks of one group
            # share kc but differ in dc, so group copies go per-(kc, dc-quad):
            # dest mht[:, dc0:dc0+4, kc...] is NOT contiguous. Instead group
            # four k-chunks? they differ in kc → dest [128, dc, 4*128] IS
            # contiguous in the last axis. So transpose the same dc for 4
            # consecutive kc into one PSUM group, then copy to
            # mht[:, dc, kc0*128:(kc0+4)*128].
            mh_nat = mres_pool.tile([P, NKC, D], f16)
            mht = mres_pool.tile([P, NDC, LKV], f16)
            mlt = mres_pool.tile([P, NDC, LKV], f16)
            for kc0 in range(0, NKC, 4):
                ml_chunks = {}
                for kc in range(kc0, kc0 + 4):
                    m_chunk = msplit_pool.tile(
                        [P, D], f32, tag="mchunk", bufs=6, name=f"mc{kc}"
                    )
                    nc.sync.dma_start(
                        out=m_chunk, in_=m_ap[kc * P:(kc + 1) * P, :]
                    )
                    # hi = fp16(M), lo = fp16(M - hi)
                    nc.vector.tensor_copy(mh_nat[:, kc, :], m_chunk)
                    ml_chunk = msplit_pool.tile(
                        [P, D], f16, tag="mlchunk", bufs=6, name=f"ml{kc}"
                    )
                    if kc % 2 == 0:
                        nc.gpsimd.tensor_sub(ml_chunk, m_chunk, mh_nat[:, kc, :])
                    else:
                        nc.vector.tensor_sub(ml_chunk, m_chunk, mh_nat[:, kc, :])
                    ml_chunks[kc] = ml_chunk
                for dc in range(NDC):
                    tg = ps_t.tile([P, 4 * P], f16, tag="tps16")
                    for j in range(4):
                        kc = kc0 + j
                        nc.tensor.transpose(
                            tg[:, j * P:(j + 1) * P],
                            mh_nat[:, kc, dc * P:(dc + 1) * P],
                            ident16,
                        )
                    if dc % 2 == 0:
                        nc.vector.tensor_copy(
                            mht[:, dc, kc0 * P:(kc0 + 4) * P], tg
                        )
                    else:
                        nc.scalar.copy(mht[:, dc, kc0 * P:(kc0 + 4) * P], tg)
                for dc in range(NDC):
                    tg = ps_t.tile([P, 4 * P], f16, tag="tps16")
                    for j in range(4):
                        kc = kc0 + j
                        nc.tensor.transpose(
                            tg[:, j * P:(j + 1) * P],
                            ml_chunks[kc][:, dc * P:(dc + 1) * P],
                            ident16,
                        )
                    if dc % 2 == 0:
                        nc.scalar.copy(mlt[:, dc, kc0 * P:(kc0 + 4) * P], tg)
                    else:
                        nc.vector.tensor_copy(
                            mlt[:, dc, kc0 * P:(kc0 + 4) * P], tg
                        )

            # ---- main loop over q tiles (Q-load/split/transpose of tile i+1
            # is emitted right after mm1(i) so the PE fills the softmax-latency
            # window with next-tile transposes).
            def load_qt(qt_i):
                """DMA q rows, scale by 50, split hi/lo fp16, PE-transpose
                into [d, q] layout."""
                q_nat = qload_pool.tile([P, D], f32, tag="qnat", name=f"qn{qt_i}")
                nc.sync.dma_start(
                    out=q_nat, in_=q_ap[qt_i * P:(qt_i + 1) * P, :]
                )
                qs = qsplit_pool.tile([P, D], f32, tag="qs", name=f"qs{qt_i}")
                nc.vector.tensor_scalar_mul(qs, q_nat, SCALE)
                qh = qsplit_pool.tile([P, D], f16, tag="qh", name=f"qh{qt_i}")
                nc.vector.tensor_copy(qh, qs)
                ql = qsplit_pool.tile([P, D], f16, tag="ql", name=f"ql{qt_i}")
                nc.vector.tensor_sub(ql, qs, qh)
                return qh, ql

            def transpose_qt(qsplit, qt_i):
                """PE-transpose the hi/lo Q into [d, q] layout; grouped
                PSUM tiles -> one [128,512] copy per 4 transposed blocks."""
                qh, ql = qsplit
                qh_t = qt_pool.tile([P, NDC, P], f16, tag="qht", name=f"qht{qt_i}")
                ql_t = qt_pool.tile([P, NDC, P], f16, tag="qlt", name=f"qlt{qt_i}")
                for src, dst, eng in ((qh, qh_t, 0), (ql, ql_t, 1)):
                    for dc0 in range(0, NDC, 4):
                        tg = ps_t.tile([P, 4 * P], f16, tag="tps16")
                        for j in range(4):
                            dc = dc0 + j
                            nc.tensor.transpose(
                                tg[:, j * P:(j + 1) * P],
                                src[:, dc * P:(dc + 1) * P],
                                ident16,
                            )
                        if (eng + dc0 // 4) % 2 == 0:
                            nc.vector.tensor_copy(dst[:, dc0:dc0 + 4, :], tg)
                        else:
                            nc.scalar.copy(dst[:, dc0:dc0 + 4, :], tg)
                return qh_t, ql_t

            qs_next = load_qt(0)
            qt_next = transpose_qt(qs_next, 0)
            for qt_i in range(NQT):
                qh_t, ql_t = qt_next
                # prefetch + split of tile i+1 runs on DVE during mm1(i)
                if qt_i + 1 < NQT:
                    qs_next = load_qt(qt_i + 1)

                # mm1: S[q, k] = (qh+ql) @ (mh+ml)^T via 3 fp16 passes,
                # accumulated in PSUM. One PSUM tile per bank so the
                # per-bank reduce_max starts as soon as that bank's
                # accumulation closes (overlapping the rest of mm1).
                s_banks = [
                    ps_s.tile([P, 512], f32, tag=f"s{ns}", name=f"s{ns}")
                    for ns in range(NS1)
                ]
                rowmax4 = vec_pool.tile([P, NS1], f32, tag="rm4")
                for ns in range(NS1):
                    sl = slice(ns * 512, (ns + 1) * 512)
                    n_mm = 3 * NDC
                    i_mm = 0
                    for lhsT, rhs in ((qh_t, mht), (qh_t, mlt), (ql_t, mht)):
                        for dc in range(NDC):
                            nc.tensor.matmul(
                                s_banks[ns],
                                lhsT=lhsT[:, dc, :],
                                rhs=rhs[:, dc, sl],
                                start=(i_mm == 0),
                                stop=(i_mm == n_mm - 1),
                            )
                            i_mm += 1
                    # per-bank row max overlaps the remaining mm1 banks
                    nc.vector.reduce_max(
                        out=rowmax4[:, ns:ns + 1], in_=s_banks[ns], axis=AX
                    )

                # next tile's Q transposes: fill the PE gap while the softmax
                # chain (last reduce + exp) runs.
                if qt_i + 1 < NQT:
                    qt_next = transpose_qt(qs_next, qt_i + 1)

                # S is already scaled by 50 (Q was), so bias is just -rowmax.
                nbias = vec_pool.tile([P, 1], f32, tag="nbias")
                nc.vector.reduce_max(out=nbias, in_=rowmax4, axis=AX, negate=True)

                p_sb = p_pool.tile([P, LKV], f16, tag="p")
                sums4 = vec_pool.tile([P, NS1], f32, tag="sm4")
                for ns in range(NS1):
                    nc.scalar.activation(
                        p_sb[:, ns * 512:(ns + 1) * 512],
                        s_banks[ns],
                        EXP,
                        bias=nbias,
                        scale=1.0,
                        accum_out=sums4[:, ns:ns + 1],
                    )
                sums = vec_pool.tile([P, 1], f32, tag="sm")
                rsum = vec_pool.tile([P, 1], f32, tag="rs")
                nc.vector.reduce_sum(out=sums, in_=sums4, axis=AX)
                nc.vector.reciprocal(rsum, sums)

                # P^T tiles (grouped copies) + mm2 (fp16)
                pt_t = pt_pool.tile([P, NKC, P], f16, tag="pt")
                o_psum = ps_o.tile([P, D], f32, tag="o")
                for kc0 in range(0, NKC, 4):
                    tg = ps_t.tile([P, 4 * P], f16, tag="tps16")
                    for j in range(4):
                        kc = kc0 + j
                        nc.tensor.transpose(
                            tg[:, j * P:(j + 1) * P],
                            p_sb[:, kc * P:(kc + 1) * P],
                            ident16,
                        )
                    if (kc0 // 4) % 2 == 0:
                        nc.vector.tensor_copy(pt_t[:, kc0:kc0 + 4, :], tg)
                    else:
                        nc.scalar.copy(pt_t[:, kc0:kc0 + 4, :], tg)
                for kc in range(NKC):
                    for ns in range(NS2):
                        nc.tensor.matmul(
                            o_psum[:, ns * 512:(ns + 1) * 512],
                            lhsT=pt_t[:, kc, :],
                            rhs=mh_nat[:, kc, ns * 512:(ns + 1) * 512],
                            start=(kc == 0),
                            stop=(kc == NKC - 1),
                        )

                # scale by 1/rowsum on ACT (Copy with per-partition scale),
                # keeping DVE free for the transpose copies.
                out_sb = out_pool.tile([P, D], f32, tag="ot")
                nc.scalar.activation(
                    out_sb, o_psum, mybir.ActivationFunctionType.Copy,
                    bias=0.0, scale=rsum,
                )
                nc.sync.dma_start(
                    out=o_ap[qt_i * P:(qt_i + 1) * P, :], in_=out_sb
                )

    return _patch_json(nc)


def get_nc():
    if "nc" not in _CACHE:
        _CACHE["nc"] = build_nc()
    return _CACHE["nc"]


def kernel(query, memory):
    from concourse.bass_utils import run_bass_kernel_spmd

    q = np.ascontiguousarray(np.asarray(query, dtype=np.float32))
    m = np.ascontiguousarray(np.asarray(memory, dtype=np.float32))
    assert q.shape == (B, LQ, D) and m.shape == (B, LKV, D)

    nc = get_nc()
    in_maps = [{"q": q[b], "m": m[b]} for b in range(B)]
    res = run_bass_kernel_spmd(nc, in_maps, core_ids=list(range(B)))
    out = np.stack([res.results[b]["out"] for b in range(B)], axis=0)
    return out
